# revision 16
# baseline (speedup 1.0000x reference)
"""Baichuan-style GQA flash-attention block (QKV proj + width-2 causal conv on
K/V + RoPE + causal attention + o_proj) on 8 Trainium2 NeuronCores.

Sharding: tensor-parallel over heads. Core c owns query heads 4c..4c+3 and kv
head c (H=32, KV=8). Each core computes its 4 heads' attention and a partial
o_proj over its 512 columns of w_o; the host sums the 8 partial outputs.

All layout prep (transposes, per-core weight slicing, rope tables) happens on
the host; the device kernel is pure matmul/vector work in a transposed
[feature, seq] layout so that the conv shift lands on the free dim and all
matmuls contract over the partition dim.

Schedule: the QKV projection streams hidden-state chunks; conv/rope/v-transpose
preprocessing is interleaved per 512-column chunk so it hides under the next
chunk's matmuls. Attention runs query-block-outer with the per-block o_proj
software-pipelined one block behind, keeping the PE stream dense end to end.
"""

import os
from contextlib import ExitStack

import numpy as np

import concourse.bacc as bacc
import concourse.mybir as mybir
import concourse.tile as tile
from concourse.bass_utils import run_bass_kernel_spmd
from concourse.masks import make_identity

B, S, HID = 1, 2048, 4096
H, KV, D = 32, 8, 128
THETA = 100000.0
NCORES = 8
HPC = H // NCORES            # query heads per core = 4
QCOLS = HPC * D              # 512
WCOLS = QCOLS + 2 * D        # 768 = q(512) | k(128) | v(128)
MCH = WCOLS // 128           # 6
NKC = HID // 128             # 32
NOB = HID // 512             # 8 o_proj column chunks
SCALE = 1.0 / float(np.sqrt(D))

F32 = mybir.dt.float32
BF16 = mybir.dt.bfloat16
F32R = mybir.dt.float32r
MULT = mybir.AluOpType.mult
ADD = mybir.AluOpType.add
EXP = mybir.ActivationFunctionType.Exp


def build_nc(S_=S, dt_mm="f32r"):
    """Build the single-core SPMD Bass program.

    dt_mm: 'f32' (exact, 4 cyc/row), 'f32r' (fp32 data, fast PE path),
           'bf16' (bf16 matmul data everywhere).
    """
    dt_d = {"f32": F32, "f32r": F32R, "bf16": BF16}[dt_mm]
    dt_stair = BF16 if dt_mm == "bf16" else F32
    NB = S_ // 512               # query blocks of 512
    SBLK = S_ // 128             # 128-blocks along seq

    nc = bacc.Bacc("TRN2", target_bir_lowering=False)
    hT = nc.dram_tensor("hT", [NKC, NB, 128, 512], dt_d, kind="ExternalInput")
    wqkvT = nc.dram_tensor("wqkvT", [HID, WCOLS], dt_d, kind="ExternalInput")
    woT = nc.dram_tensor("woT", [HPC, D, HID], dt_d, kind="ExternalInput")
    cosF = nc.dram_tensor("cosF", [128, S_], F32, kind="ExternalInput")
    sinF = nc.dram_tensor("sinF", [128, S_], F32, kind="ExternalInput")
    convk = nc.dram_tensor("convk", [128, 2], F32, kind="ExternalInput")
    convv = nc.dram_tensor("convv", [128, 2], F32, kind="ExternalInput")
    ones_c = nc.dram_tensor("ones_c", [128, 1], dt_d, kind="ExternalInput")
    ones_r = nc.dram_tensor("ones_r", [1, 128], F32, kind="ExternalInput")
    outp = nc.dram_tensor("outp", [NOB, SBLK, 128, 512], F32,
                          kind="ExternalOutput")

    with tile.TileContext(nc) as tc, ExitStack() as top:
        persist = top.enter_context(tc.tile_pool(name="persist", bufs=1))
        # lower-left triangle: keep where col >= row
        stair = persist.tile([128, 128], dt_stair, tag="stair")
        nc.gpsimd.memset(stair[:], 1.0)
        nc.gpsimd.affine_select(
            out=stair[:], in_=stair[:], compare_op=mybir.AluOpType.is_ge,
            fill=0.0, base=0, pattern=[[1, 128]], channel_multiplier=-1)
        ones_col = persist.tile([128, 1], dt_d, tag="ones_col")
        nc.gpsimd.dma_start(ones_col[:], ones_c[:])
        ones_row = persist.tile([1, 128], F32, tag="ones_row")
        nc.gpsimd.dma_start(ones_row[:], ones_r[:])

        res1 = top.enter_context(tc.tile_pool(name="res1", bufs=1))
        qTr = [res1.tile([128, S_], dt_d, tag=f"qTr{h}", name=f"qTr{h}")
               for h in range(HPC)]
        kTr = res1.tile([128, S_], dt_d, tag="kTr")
        vS = res1.tile([128, S_], dt_d, tag="vS")

        # ============ phase 1: QKV projection + fused preproc ============
        with ExitStack() as ph1:
            trig = ph1.enter_context(tc.tile_pool(name="trig", bufs=1))
            wp = ph1.enter_context(tc.tile_pool(name="wp", bufs=1))
            hp = ph1.enter_context(tc.tile_pool(name="hp", bufs=4))
            ckp = ph1.enter_context(tc.tile_pool(name="ckp", bufs=12))
            tp = ph1.enter_context(tc.tile_pool(name="tp", bufs=2))
            psq = ph1.enter_context(tc.tile_pool(name="psq", bufs=6,
                                                 space="PSUM"))
            pst = ph1.enter_context(tc.tile_pool(name="pst", bufs=2,
                                                 space="PSUM"))
            ck_sb = trig.tile([128, 2], F32, tag="ck_sb")
            cv_sb = trig.tile([128, 2], F32, tag="cv_sb")
            nc.gpsimd.dma_start(ck_sb[:], convk[:])
            nc.gpsimd.dma_start(cv_sb[:], convv[:])
            ident = trig.tile([128, 128], F32, tag="ident")
            make_identity(nc, ident[:])
            cos_sb = trig.tile([128, S_], F32, tag="cos_sb")
            sin_sb = trig.tile([128, S_], F32, tag="sin_sb")
            nc.gpsimd.dma_start(cos_sb[:], cosF[:])
            nc.gpsimd.dma_start(sin_sb[:], sinF[:])

            def rope_chunk(dst, x, n):
                """dst[:, n*512:+512] = rope(x); x is [128,512] fp32 chunk."""
                sl = slice(n * 512, (n + 1) * 512)
                t1 = tp.tile([128, 512], F32, tag="r1", name=f"r1_{n}")
                t2 = tp.tile([128, 512], F32, tag="r2", name=f"r2_{n}")
                nc.vector.tensor_copy(t2[0:64, :], x[64:128, :])
                nc.vector.tensor_copy(t2[64:128, :], x[0:64, :])
                nc.vector.tensor_tensor(t1[:], x[:], cos_sb[:, sl], MULT)
                nc.vector.tensor_tensor(t2[:], t2[:], sin_sb[:, sl], MULT)
                nc.vector.tensor_tensor(dst[:, sl], t1[:], t2[:], ADD)

            def conv_chunk(x, xprev, cc, n):
                """width-2 causal conv on a [128,512] chunk; returns tile."""
                t2 = tp.tile([128, 512], F32, tag="r2", name=f"cv2_{n}")
                dst = tp.tile([128, 512], F32, tag="cv1", name=f"cv1_{n}")
                nc.vector.tensor_scalar_mul(t2[:], x[:], cc[:, 0:1])
                nc.vector.scalar_tensor_tensor(
                    out=dst[:, 1:512], in0=x[:, 1:512], scalar=cc[:, 1:2],
                    in1=t2[:, 0:511], op0=MULT, op1=ADD)
                if xprev is None:
                    nc.vector.tensor_scalar_mul(dst[:, 0:1], x[:, 0:1],
                                                cc[:, 1:2])
                else:
                    tb = tp.tile([128, 1], F32, tag="cvp", name=f"cvp_{n}")
                    nc.vector.tensor_scalar_mul(tb[:], xprev[:, 511:512],
                                                cc[:, 0:1])
                    nc.vector.scalar_tensor_tensor(
                        out=dst[:, 0:1], in0=x[:, 0:1], scalar=cc[:, 1:2],
                        in1=tb[:], op0=MULT, op1=ADD)
                return dst

            w_sb = [wp.tile([128, WCOLS], dt_d, tag=f"w{k}", name=f"w{k}")
                    for k in range(NKC)]
            kprev = [None, None]  # previous raw chunks of k, v for conv edge
            for n in range(NB):
                psums = [psq.tile([128, 512], F32, tag="qkps",
                                  name=f"qkps{n}_{m}") for m in range(MCH)]
                for k in range(NKC):
                    if n == 0:
                        weng = nc.scalar if k % 2 == 0 else nc.sync
                        weng.dma_start(w_sb[k][:],
                                       wqkvT[k * 128:(k + 1) * 128, :])
                    ht = hp.tile([128, 512], dt_d, tag="ht", name=f"ht{n}_{k}")
                    heng = nc.sync if k % 2 == 0 else nc.scalar
                    heng.dma_start(ht[:], hT[k, n, :, :])
                    for m in range(MCH):
                        nc.tensor.matmul(
                            psums[m][:], lhsT=w_sb[k][:, m * 128:(m + 1) * 128],
                            rhs=ht[:], start=(k == 0), stop=(k == NKC - 1))
                chunks = [None] * MCH

                def evac(m):
                    ch = ckp.tile([128, 512], F32, tag="qkvch",
                                  name=f"ch{n}_{m}")
                    nc.vector.tensor_copy(ch[:], psums[m][:])
                    chunks[m] = ch

                # evac + preproc in dependency-first order: k, q0 unblock the
                # next phase's first scores matmuls
                evac(HPC)
                kc = conv_chunk(chunks[HPC], kprev[0], ck_sb, n)
                rope_chunk(kTr, kc, n)
                evac(0)
                rope_chunk(qTr[0], chunks[0], n)
                evac(HPC + 1)
                vc = conv_chunk(chunks[HPC + 1], kprev[1], cv_sb, n)
                for jj in range(4):
                    jb = 4 * n + jj
                    pt = pst.tile([128, 128], F32, tag="pt", name=f"pt{jb}")
                    nc.tensor.transpose(pt[:], vc[:, jj * 128:(jj + 1) * 128],
                                        ident[:])
                    nc.vector.tensor_copy(vS[:, jb * 128:(jb + 1) * 128],
                                          pt[:])
                for h in range(1, HPC):
                    evac(h)
                    rope_chunk(qTr[h], chunks[h], n)
                kprev = [chunks[HPC], chunks[HPC + 1]]

        # ============ phases 2+3: attention + pipelined o_proj ===========
        with ExitStack() as ph34:
            big = ph34.enter_context(tc.tile_pool(name="big", bufs=1))
            exp_p = ph34.enter_context(tc.tile_pool(name="exp_p", bufs=4))
            smalls = ph34.enter_context(tc.tile_pool(name="smalls", bufs=2))
            stg = ph34.enter_context(tc.tile_pool(name="stg", bufs=4))
            ps_s = ph34.enter_context(tc.tile_pool(name="ps_s", bufs=2,
                                                   space="PSUM"))
            ps_o = ph34.enter_context(tc.tile_pool(name="ps_o", bufs=2,
                                                   space="PSUM"))
            ps_d = ph34.enter_context(tc.tile_pool(name="ps_d", bufs=1,
                                                   space="PSUM"))
            ps_b = ph34.enter_context(tc.tile_pool(name="ps_b", bufs=1,
                                                   space="PSUM"))
            ps_p = ph34.enter_context(tc.tile_pool(name="ps_p", bufs=2,
                                                   space="PSUM"))
            AT = [big.tile([128, S_], dt_d, tag=f"AT{h}", name=f"AT{h}")
                  for h in range(HPC)]
            woT_sb = []
            for h in range(HPC):
                wt = big.tile([128, HID], dt_d, tag=f"woT{h}", name=f"woT{h}")
                nc.scalar.dma_start(wt[:], woT[h, :, :])
                woT_sb.append(wt)

            def oproj_block(bi):
                for nb in range(NOB):
                    for sb in range(4 * bi, 4 * bi + 4):
                        pp = ps_p.tile([128, 512], F32, tag="pp",
                                       name=f"pp{nb}_{sb}")
                        for h in range(HPC):
                            nc.tensor.matmul(
                                pp[:], lhsT=AT[h][:, sb * 128:(sb + 1) * 128],
                                rhs=woT_sb[h][:, nb * 512:(nb + 1) * 512],
                                start=(h == 0), stop=(h == HPC - 1))
                        ot = stg.tile([128, 512], F32, tag="ot",
                                      name=f"ot{nb}_{sb}")
                        nc.vector.tensor_copy(ot[:], pp[:])
                        oeng = nc.sync if (nb + sb) % 2 == 0 else nc.gpsimd
                        oeng.dma_start(outp[nb, sb, :, :], ot[:])

            for bi in range(NB):
                norm_jobs = []
                for h in range(HPC):
                    po = ps_o.tile([128, 512], F32, tag="po", name=f"po{h}_{bi}")
                    pd = ps_d.tile([1, 512], F32, tag="pd", name=f"pd{h}_{bi}")
                    nj = 4 * bi + 4
                    for jb in range(nj):
                        t = jb - 4 * bi
                        lo = 128 * t if t > 0 else 0   # valid query range start
                        ps = ps_s.tile([128, 512], F32, tag="ps",
                                       name=f"ps{h}_{bi}_{jb}")
                        nc.tensor.matmul(
                            ps[:, lo:512],
                            lhsT=kTr[:, jb * 128:(jb + 1) * 128],
                            rhs=qTr[h][:, bi * 512 + lo:(bi + 1) * 512],
                            start=True, stop=True)
                        ex = exp_p.tile([128, 512], dt_d, tag="ex",
                                        name=f"ex{h}_{bi}_{jb}")
                        nc.scalar.activation(ex[:, lo:512], ps[:, lo:512],
                                             EXP, scale=SCALE)
                        if t >= 0:
                            nc.vector.tensor_tensor(
                                ex[:, 128 * t:128 * t + 128],
                                ex[:, 128 * t:128 * t + 128], stair[:], MULT)
                        nc.tensor.matmul(po[:, lo:512],
                                         lhsT=vS[:, jb * 128:(jb + 1) * 128],
                                         rhs=ex[:, lo:512], start=(jb == 0),
                                         stop=(jb == nj - 1))
                        nc.tensor.matmul(pd[:, lo:512], lhsT=ones_col[:],
                                         rhs=ex[:, lo:512], start=(jb == 0),
                                         stop=(jb == nj - 1))
                    norm_jobs.append((h, po, pd))
                # normalization, emitted after all heads so the PE never
                # waits on the DVE reciprocal chain
                for h, po, pd in norm_jobs:
                    dsb = smalls.tile([1, 512], F32, tag="dsb",
                                      name=f"dsb{h}_{bi}")
                    nc.vector.tensor_copy(dsb[:], pd[:])
                    rec = smalls.tile([1, 512], F32, tag="rec",
                                      name=f"rec{h}_{bi}")
                    scr = smalls.tile([1, 512], F32, tag="scr",
                                      name=f"scr{h}_{bi}")
                    nc.vector.reciprocal_approx_accurate(rec[:], dsb[:],
                                                         scr[:])
                    pb = ps_b.tile([128, 512], F32, tag="pb",
                                   name=f"pb{h}_{bi}")
                    nc.tensor.matmul(pb[:], lhsT=ones_row[:], rhs=rec[:],
                                     start=True, stop=True)
                    pbs = smalls.tile([128, 512], F32, tag="pbs",
                                      name=f"pbs{h}_{bi}")
                    nc.vector.tensor_copy(pbs[:], pb[:])
                    nc.vector.tensor_tensor(AT[h][:, bi * 512:(bi + 1) * 512],
                                            po[:], pbs[:], MULT)
                if bi > 0:
                    oproj_block(bi - 1)
            oproj_block(NB - 1)

    nc.compile()
    return nc


def host_prep(hidden_states, w_pack, w_o, conv_k, conv_v, S_=S, dt_mm="f32r"):
    """Slice/transpose full inputs into 8 per-core input maps."""
    if dt_mm == "bf16":
        import ml_dtypes
        dt_np = ml_dtypes.bfloat16
    else:
        dt_np = np.float32

    hid2 = np.asarray(hidden_states).reshape(S_, HID)
    # chunk-major pack: [NKC, NB, 128, 512] so each DMA tile is contiguous
    hTv = np.ascontiguousarray(
        hid2.T.reshape(NKC, 128, S_ // 512, 512).transpose(0, 2, 1, 3)
    ).astype(dt_np, copy=False)

    inv = 1.0 / (THETA ** (np.arange(0, D, 2, dtype=np.float32) / D))  # [64]
    t = np.arange(S_, dtype=np.float32)
    freqs = t[:, None] * inv[None, :]                                   # [S, 64]
    cos = np.cos(freqs).T.astype(np.float32)                            # [64, S]
    sin = np.sin(freqs).T.astype(np.float32)
    cosF = np.ascontiguousarray(np.concatenate([cos, cos], 0))          # [128, S]
    sinF = np.ascontiguousarray(np.concatenate([-sin, sin], 0))

    w_pack = np.asarray(w_pack)
    w_o = np.asarray(w_o)
    conv_k = np.asarray(conv_k)
    conv_v = np.asarray(conv_v)

    in_maps = []
    for c in range(NCORES):
        qw = w_pack[c * QCOLS:(c + 1) * QCOLS]                 # [512, HID]
        kw = w_pack[H * D + c * D:H * D + (c + 1) * D]         # [128, HID]
        vw = w_pack[H * D + KV * D + c * D:H * D + KV * D + (c + 1) * D]
        wqkvT = np.ascontiguousarray(
            np.concatenate([qw, kw, vw], 0).T).astype(dt_np, copy=False)
        woT = np.ascontiguousarray(
            w_o[:, c * QCOLS:(c + 1) * QCOLS].T.reshape(HPC, D, HID)
        ).astype(dt_np, copy=False)
        in_maps.append(dict(
            hT=hTv, wqkvT=wqkvT, woT=woT, cosF=cosF, sinF=sinF,
            ones_c=np.ones((128, 1), dt_np),
            ones_r=np.ones((1, 128), np.float32),
            convk=np.ascontiguousarray(
                np.broadcast_to(conv_k[c], (128, 2))).astype(np.float32),
            convv=np.ascontiguousarray(
                np.broadcast_to(conv_v[c], (128, 2))).astype(np.float32),
        ))
    return in_maps


def gather(results):
    """Sum per-core chunk-major partials -> full [B, S, HID] output."""
    acc = results[0]["outp"].astype(np.float32)
    for c in range(1, NCORES):
        acc = acc + results[c]["outp"]
    # [NOB, SBLK, 128, 512] chunk-major -> [S, HID]
    out = acc.transpose(1, 2, 0, 3).reshape(S, HID)
    return np.ascontiguousarray(out).reshape(B, S, HID)


_NC_CACHE = {}


def _get_nc(S_=S, dt_mm="f32r"):
    key = (S_, dt_mm)
    if key not in _NC_CACHE:
        _NC_CACHE[key] = build_nc(S_, dt_mm)
    return _NC_CACHE[key]


def kernel(**inputs):
    dt_mm = os.environ.get("KERNEL_DT_MM", "f32r")
    nc = _get_nc(S, dt_mm)
    in_maps = host_prep(
        inputs["hidden_states"], inputs["w_pack"], inputs["w_o"],
        inputs["conv_k"], inputs["conv_v"], S, dt_mm)
    res = run_bass_kernel_spmd(nc, in_maps, core_ids=list(range(NCORES)))
    return gather(res.results)


# revision 17
# speedup vs baseline: 1.1286x; 1.1286x over previous
"""Baichuan-style GQA flash-attention block (QKV proj + width-2 causal conv on
K/V + RoPE + causal attention + o_proj) on 8 Trainium2 NeuronCores.

Sharding: tensor-parallel over heads. Core c owns query heads 4c..4c+3 and kv
head c (H=32, KV=8). Each core computes its 4 heads' attention and a partial
o_proj over its 512 columns of w_o; the host sums the 8 partial outputs.

All layout prep (transposes, per-core weight slicing, rope tables) happens on
the host; the device kernel is pure matmul/vector work in a transposed
[feature, seq] layout so that the conv shift lands on the free dim and all
matmuls contract over the partition dim.

Schedule: the QKV projection streams hidden-state chunks; conv/rope/v-transpose
preprocessing is interleaved per 512-column chunk so it hides under the next
chunk's matmuls. Attention runs query-block-outer with the per-block o_proj
software-pipelined one block behind, keeping the PE stream dense end to end.
"""

import os
from contextlib import ExitStack

import numpy as np

import concourse.bacc as bacc
import concourse.mybir as mybir
import concourse.tile as tile
from concourse.bass_utils import run_bass_kernel_spmd
from concourse.masks import make_identity

B, S, HID = 1, 2048, 4096
H, KV, D = 32, 8, 128
THETA = 100000.0
NCORES = 8
HPC = H // NCORES            # query heads per core = 4
QCOLS = HPC * D              # 512
WCOLS = QCOLS + 2 * D        # 768 = q(512) | k(128) | v(128)
MCH = WCOLS // 128           # 6
NKC = HID // 128             # 32
NOB = HID // 512             # 8 o_proj column chunks
SCALE = 1.0 / float(np.sqrt(D))

F32 = mybir.dt.float32
BF16 = mybir.dt.bfloat16
F32R = mybir.dt.float32r
MULT = mybir.AluOpType.mult
ADD = mybir.AluOpType.add
EXP = mybir.ActivationFunctionType.Exp


def build_nc(S_=S, dt_mm="f32r"):
    """Build the single-core SPMD Bass program.

    dt_mm: 'f32' (exact, 4 cyc/row), 'f32r' (fp32 data, fast PE path),
           'bf16' (bf16 matmul data everywhere).
    """
    dt_d = {"f32": F32, "f32r": F32R, "bf16": BF16}[dt_mm]
    dt_stair = BF16 if dt_mm == "bf16" else F32
    NB = S_ // 512               # query blocks of 512
    SBLK = S_ // 128             # 128-blocks along seq

    nc = bacc.Bacc("TRN2", target_bir_lowering=False)
    hT = nc.dram_tensor("hT", [NKC, NB, 128, 512], dt_d, kind="ExternalInput")
    wqkvT = nc.dram_tensor("wqkvT", [HID, WCOLS], dt_d, kind="ExternalInput")
    woT = nc.dram_tensor("woT", [HPC, D, HID], dt_d, kind="ExternalInput")
    cosF = nc.dram_tensor("cosF", [128, S_], F32, kind="ExternalInput")
    sinF = nc.dram_tensor("sinF", [128, S_], F32, kind="ExternalInput")
    convk = nc.dram_tensor("convk", [128, 2], F32, kind="ExternalInput")
    convv = nc.dram_tensor("convv", [128, 2], F32, kind="ExternalInput")
    ones_c = nc.dram_tensor("ones_c", [128, 1], dt_d, kind="ExternalInput")
    ones_r = nc.dram_tensor("ones_r", [1, 128], F32, kind="ExternalInput")
    outp = nc.dram_tensor("outp", [NOB, SBLK, 128, 512], F32,
                          kind="ExternalOutput")

    with tile.TileContext(nc) as tc, ExitStack() as top:
        persist = top.enter_context(tc.tile_pool(name="persist", bufs=1))
        # lower-left triangle: keep where col >= row
        stair = persist.tile([128, 128], dt_stair, tag="stair")
        nc.gpsimd.memset(stair[:], 1.0)
        nc.gpsimd.affine_select(
            out=stair[:], in_=stair[:], compare_op=mybir.AluOpType.is_ge,
            fill=0.0, base=0, pattern=[[1, 128]], channel_multiplier=-1)
        ones_col = persist.tile([128, 1], dt_d, tag="ones_col")
        nc.gpsimd.dma_start(ones_col[:], ones_c[:])
        ones_row = persist.tile([1, 128], F32, tag="ones_row")
        nc.gpsimd.dma_start(ones_row[:], ones_r[:])

        res1 = top.enter_context(tc.tile_pool(name="res1", bufs=1))
        qTr = [res1.tile([128, S_], dt_d, tag=f"qTr{h}", name=f"qTr{h}")
               for h in range(HPC)]
        kTr = res1.tile([128, S_], dt_d, tag="kTr")
        vS = res1.tile([128, S_], dt_d, tag="vS")

        # ============ phase 1: QKV projection + fused preproc ============
        with ExitStack() as ph1:
            trig = ph1.enter_context(tc.tile_pool(name="trig", bufs=1))
            wp = ph1.enter_context(tc.tile_pool(name="wp", bufs=1))
            hp = ph1.enter_context(tc.tile_pool(name="hp", bufs=4))
            ckp = ph1.enter_context(tc.tile_pool(name="ckp", bufs=12))
            tp = ph1.enter_context(tc.tile_pool(name="tp", bufs=2))
            psq = ph1.enter_context(tc.tile_pool(name="psq", bufs=6,
                                                 space="PSUM"))
            pst = ph1.enter_context(tc.tile_pool(name="pst", bufs=2,
                                                 space="PSUM"))
            ck_sb = trig.tile([128, 2], F32, tag="ck_sb")
            cv_sb = trig.tile([128, 2], F32, tag="cv_sb")
            nc.gpsimd.dma_start(ck_sb[:], convk[:])
            nc.gpsimd.dma_start(cv_sb[:], convv[:])
            ident = trig.tile([128, 128], F32, tag="ident")
            make_identity(nc, ident[:])
            cos_sb = trig.tile([128, S_], F32, tag="cos_sb")
            sin_sb = trig.tile([128, S_], F32, tag="sin_sb")
            nc.gpsimd.dma_start(cos_sb[:], cosF[:])
            nc.gpsimd.dma_start(sin_sb[:], sinF[:])

            def rope_chunk(dst, x, n):
                """dst[:, n*512:+512] = rope(x); x is [128,512] fp32 chunk."""
                sl = slice(n * 512, (n + 1) * 512)
                t1 = tp.tile([128, 512], F32, tag="r1", name=f"r1_{n}")
                t2 = tp.tile([128, 512], F32, tag="r2", name=f"r2_{n}")
                nc.vector.tensor_copy(t2[0:64, :], x[64:128, :])
                nc.vector.tensor_copy(t2[64:128, :], x[0:64, :])
                nc.vector.tensor_tensor(t1[:], x[:], cos_sb[:, sl], MULT)
                nc.vector.tensor_tensor(t2[:], t2[:], sin_sb[:, sl], MULT)
                nc.vector.tensor_tensor(dst[:, sl], t1[:], t2[:], ADD)

            def conv_chunk(x, xprev, cc, n):
                """width-2 causal conv on a [128,512] chunk; returns tile."""
                t2 = tp.tile([128, 512], F32, tag="r2", name=f"cv2_{n}")
                dst = tp.tile([128, 512], F32, tag="cv1", name=f"cv1_{n}")
                nc.vector.tensor_scalar_mul(t2[:], x[:], cc[:, 0:1])
                nc.vector.scalar_tensor_tensor(
                    out=dst[:, 1:512], in0=x[:, 1:512], scalar=cc[:, 1:2],
                    in1=t2[:, 0:511], op0=MULT, op1=ADD)
                if xprev is None:
                    nc.vector.tensor_scalar_mul(dst[:, 0:1], x[:, 0:1],
                                                cc[:, 1:2])
                else:
                    tb = tp.tile([128, 1], F32, tag="cvp", name=f"cvp_{n}")
                    nc.vector.tensor_scalar_mul(tb[:], xprev[:, 511:512],
                                                cc[:, 0:1])
                    nc.vector.scalar_tensor_tensor(
                        out=dst[:, 0:1], in0=x[:, 0:1], scalar=cc[:, 1:2],
                        in1=tb[:], op0=MULT, op1=ADD)
                return dst

            w_sb = [wp.tile([128, WCOLS], dt_d, tag=f"w{k}", name=f"w{k}")
                    for k in range(NKC)]
            kprev = [None, None]  # previous raw chunks of k, v for conv edge
            for n in range(NB):
                psums = [psq.tile([128, 512], F32, tag="qkps",
                                  name=f"qkps{n}_{m}") for m in range(MCH)]
                for k in range(NKC):
                    if n == 0:
                        weng = nc.scalar if k % 2 == 0 else nc.sync
                        weng.dma_start(w_sb[k][:],
                                       wqkvT[k * 128:(k + 1) * 128, :])
                    ht = hp.tile([128, 512], dt_d, tag="ht", name=f"ht{n}_{k}")
                    heng = nc.sync if k % 2 == 0 else nc.scalar
                    heng.dma_start(ht[:], hT[k, n, :, :])
                    for m in range(MCH):
                        nc.tensor.matmul(
                            psums[m][:], lhsT=w_sb[k][:, m * 128:(m + 1) * 128],
                            rhs=ht[:], start=(k == 0), stop=(k == NKC - 1))
                chunks = [None] * MCH

                def evac(m):
                    ch = ckp.tile([128, 512], F32, tag="qkvch",
                                  name=f"ch{n}_{m}")
                    nc.vector.tensor_copy(ch[:], psums[m][:])
                    chunks[m] = ch

                # Evacuate PSUM banks promptly so the next chunk's matmuls
                # get banks back; on the last chunk interleave dependency-
                # first (k, q0 early) to shorten the phase transition.
                if n < NB - 1:
                    for m in (HPC, 0, HPC + 1, 1, 2, 3):
                        evac(m)
                    kc = conv_chunk(chunks[HPC], kprev[0], ck_sb, n)
                    rope_chunk(kTr, kc, n)
                    rope_chunk(qTr[0], chunks[0], n)
                    vc = conv_chunk(chunks[HPC + 1], kprev[1], cv_sb, n)
                    for jj in range(4):
                        jb = 4 * n + jj
                        pt = pst.tile([128, 128], F32, tag="pt",
                                      name=f"pt{jb}")
                        nc.tensor.transpose(
                            pt[:], vc[:, jj * 128:(jj + 1) * 128], ident[:])
                        nc.vector.tensor_copy(
                            vS[:, jb * 128:(jb + 1) * 128], pt[:])
                    for h in range(1, HPC):
                        rope_chunk(qTr[h], chunks[h], n)
                else:
                    evac(HPC)
                    kc = conv_chunk(chunks[HPC], kprev[0], ck_sb, n)
                    rope_chunk(kTr, kc, n)
                    evac(0)
                    rope_chunk(qTr[0], chunks[0], n)
                    evac(HPC + 1)
                    vc = conv_chunk(chunks[HPC + 1], kprev[1], cv_sb, n)
                    for jj in range(4):
                        jb = 4 * n + jj
                        pt = pst.tile([128, 128], F32, tag="pt",
                                      name=f"pt{jb}")
                        nc.tensor.transpose(
                            pt[:], vc[:, jj * 128:(jj + 1) * 128], ident[:])
                        nc.vector.tensor_copy(
                            vS[:, jb * 128:(jb + 1) * 128], pt[:])
                    for h in range(1, HPC):
                        evac(h)
                        rope_chunk(qTr[h], chunks[h], n)
                kprev = [chunks[HPC], chunks[HPC + 1]]

        # ============ phases 2+3: attention + pipelined o_proj ===========
        with ExitStack() as ph34:
            big = ph34.enter_context(tc.tile_pool(name="big", bufs=1))
            exp_p = ph34.enter_context(tc.tile_pool(name="exp_p", bufs=4))
            smalls = ph34.enter_context(tc.tile_pool(name="smalls", bufs=2))
            stg = ph34.enter_context(tc.tile_pool(name="stg", bufs=4))
            ps_s = ph34.enter_context(tc.tile_pool(name="ps_s", bufs=2,
                                                   space="PSUM"))
            ps_o = ph34.enter_context(tc.tile_pool(name="ps_o", bufs=2,
                                                   space="PSUM"))
            ps_d = ph34.enter_context(tc.tile_pool(name="ps_d", bufs=1,
                                                   space="PSUM"))
            ps_b = ph34.enter_context(tc.tile_pool(name="ps_b", bufs=1,
                                                   space="PSUM"))
            ps_p = ph34.enter_context(tc.tile_pool(name="ps_p", bufs=2,
                                                   space="PSUM"))
            AT = [big.tile([128, S_], dt_d, tag=f"AT{h}", name=f"AT{h}")
                  for h in range(HPC)]
            woT_sb = []
            for h in range(HPC):
                wt = big.tile([128, HID], dt_d, tag=f"woT{h}", name=f"woT{h}")
                nc.scalar.dma_start(wt[:], woT[h, :, :])
                woT_sb.append(wt)

            def oproj_block(bi):
                for nb in range(NOB):
                    for sb in range(4 * bi, 4 * bi + 4):
                        pp = ps_p.tile([128, 512], F32, tag="pp",
                                       name=f"pp{nb}_{sb}")
                        for h in range(HPC):
                            nc.tensor.matmul(
                                pp[:], lhsT=AT[h][:, sb * 128:(sb + 1) * 128],
                                rhs=woT_sb[h][:, nb * 512:(nb + 1) * 512],
                                start=(h == 0), stop=(h == HPC - 1))
                        ot = stg.tile([128, 512], F32, tag="ot",
                                      name=f"ot{nb}_{sb}")
                        nc.vector.tensor_copy(ot[:], pp[:])
                        oeng = nc.sync if (nb + sb) % 2 == 0 else nc.gpsimd
                        oeng.dma_start(outp[nb, sb, :, :], ot[:])

            for bi in range(NB):
                norm_jobs = []
                for h in range(HPC):
                    po = ps_o.tile([128, 512], F32, tag="po", name=f"po{h}_{bi}")
                    pd = ps_d.tile([1, 512], F32, tag="pd", name=f"pd{h}_{bi}")
                    nj = 4 * bi + 4
                    for jb in range(nj):
                        t = jb - 4 * bi
                        lo = 128 * t if t > 0 else 0   # valid query range start
                        ps = ps_s.tile([128, 512], F32, tag="ps",
                                       name=f"ps{h}_{bi}_{jb}")
                        nc.tensor.matmul(
                            ps[:, lo:512],
                            lhsT=kTr[:, jb * 128:(jb + 1) * 128],
                            rhs=qTr[h][:, bi * 512 + lo:(bi + 1) * 512],
                            start=True, stop=True)
                        ex = exp_p.tile([128, 512], dt_d, tag="ex",
                                        name=f"ex{h}_{bi}_{jb}")
                        nc.scalar.activation(ex[:, lo:512], ps[:, lo:512],
                                             EXP, scale=SCALE)
                        if t >= 0:
                            nc.vector.tensor_tensor(
                                ex[:, 128 * t:128 * t + 128],
                                ex[:, 128 * t:128 * t + 128], stair[:], MULT)
                        nc.tensor.matmul(po[:, lo:512],
                                         lhsT=vS[:, jb * 128:(jb + 1) * 128],
                                         rhs=ex[:, lo:512], start=(jb == 0),
                                         stop=(jb == nj - 1))
                        nc.tensor.matmul(pd[:, lo:512], lhsT=ones_col[:],
                                         rhs=ex[:, lo:512], start=(jb == 0),
                                         stop=(jb == nj - 1))
                    norm_jobs.append((h, po, pd))
                # normalization, emitted after all heads so the PE never
                # waits on the DVE reciprocal chain
                for h, po, pd in norm_jobs:
                    dsb = smalls.tile([1, 512], F32, tag="dsb",
                                      name=f"dsb{h}_{bi}")
                    nc.vector.tensor_copy(dsb[:], pd[:])
                    rec = smalls.tile([1, 512], F32, tag="rec",
                                      name=f"rec{h}_{bi}")
                    scr = smalls.tile([1, 512], F32, tag="scr",
                                      name=f"scr{h}_{bi}")
                    nc.vector.reciprocal_approx_accurate(rec[:], dsb[:],
                                                         scr[:])
                    pb = ps_b.tile([128, 512], F32, tag="pb",
                                   name=f"pb{h}_{bi}")
                    nc.tensor.matmul(pb[:], lhsT=ones_row[:], rhs=rec[:],
                                     start=True, stop=True)
                    pbs = smalls.tile([128, 512], F32, tag="pbs",
                                      name=f"pbs{h}_{bi}")
                    nc.vector.tensor_copy(pbs[:], pb[:])
                    nc.vector.tensor_tensor(AT[h][:, bi * 512:(bi + 1) * 512],
                                            po[:], pbs[:], MULT)
                if bi > 0:
                    oproj_block(bi - 1)
            oproj_block(NB - 1)

    nc.compile()
    return nc


def host_prep(hidden_states, w_pack, w_o, conv_k, conv_v, S_=S, dt_mm="f32r"):
    """Slice/transpose full inputs into 8 per-core input maps."""
    if dt_mm == "bf16":
        import ml_dtypes
        dt_np = ml_dtypes.bfloat16
    else:
        dt_np = np.float32

    hid2 = np.asarray(hidden_states).reshape(S_, HID)
    # chunk-major pack: [NKC, NB, 128, 512] so each DMA tile is contiguous
    hTv = np.ascontiguousarray(
        hid2.T.reshape(NKC, 128, S_ // 512, 512).transpose(0, 2, 1, 3)
    ).astype(dt_np, copy=False)

    inv = 1.0 / (THETA ** (np.arange(0, D, 2, dtype=np.float32) / D))  # [64]
    t = np.arange(S_, dtype=np.float32)
    freqs = t[:, None] * inv[None, :]                                   # [S, 64]
    cos = np.cos(freqs).T.astype(np.float32)                            # [64, S]
    sin = np.sin(freqs).T.astype(np.float32)
    cosF = np.ascontiguousarray(np.concatenate([cos, cos], 0))          # [128, S]
    sinF = np.ascontiguousarray(np.concatenate([-sin, sin], 0))

    w_pack = np.asarray(w_pack)
    w_o = np.asarray(w_o)
    conv_k = np.asarray(conv_k)
    conv_v = np.asarray(conv_v)

    in_maps = []
    for c in range(NCORES):
        qw = w_pack[c * QCOLS:(c + 1) * QCOLS]                 # [512, HID]
        kw = w_pack[H * D + c * D:H * D + (c + 1) * D]         # [128, HID]
        vw = w_pack[H * D + KV * D + c * D:H * D + KV * D + (c + 1) * D]
        wqkvT = np.ascontiguousarray(
            np.concatenate([qw, kw, vw], 0).T).astype(dt_np, copy=False)
        woT = np.ascontiguousarray(
            w_o[:, c * QCOLS:(c + 1) * QCOLS].T.reshape(HPC, D, HID)
        ).astype(dt_np, copy=False)
        in_maps.append(dict(
            hT=hTv, wqkvT=wqkvT, woT=woT, cosF=cosF, sinF=sinF,
            ones_c=np.ones((128, 1), dt_np),
            ones_r=np.ones((1, 128), np.float32),
            convk=np.ascontiguousarray(
                np.broadcast_to(conv_k[c], (128, 2))).astype(np.float32),
            convv=np.ascontiguousarray(
                np.broadcast_to(conv_v[c], (128, 2))).astype(np.float32),
        ))
    return in_maps


def gather(results):
    """Sum per-core chunk-major partials -> full [B, S, HID] output."""
    acc = results[0]["outp"].astype(np.float32)
    for c in range(1, NCORES):
        acc = acc + results[c]["outp"]
    # [NOB, SBLK, 128, 512] chunk-major -> [S, HID]
    out = acc.transpose(1, 2, 0, 3).reshape(S, HID)
    return np.ascontiguousarray(out).reshape(B, S, HID)


_NC_CACHE = {}


def _get_nc(S_=S, dt_mm="f32r"):
    key = (S_, dt_mm)
    if key not in _NC_CACHE:
        _NC_CACHE[key] = build_nc(S_, dt_mm)
    return _NC_CACHE[key]


def kernel(**inputs):
    dt_mm = os.environ.get("KERNEL_DT_MM", "f32r")
    nc = _get_nc(S, dt_mm)
    in_maps = host_prep(
        inputs["hidden_states"], inputs["w_pack"], inputs["w_o"],
        inputs["conv_k"], inputs["conv_v"], S, dt_mm)
    res = run_bass_kernel_spmd(nc, in_maps, core_ids=list(range(NCORES)))
    return gather(res.results)


# revision 18
# speedup vs baseline: 1.1508x; 1.0197x over previous
"""Baichuan-style GQA flash-attention block (QKV proj + width-2 causal conv on
K/V + RoPE + causal attention + o_proj) on 8 Trainium2 NeuronCores.

Sharding: tensor-parallel over heads. Core c owns query heads 4c..4c+3 and kv
head c (H=32, KV=8). Each core computes its 4 heads' attention and a partial
o_proj over its 512 columns of w_o; the host sums the 8 partial outputs.

All layout prep (transposes, per-core weight slicing, rope tables) happens on
the host; the device kernel is pure matmul/vector work in a transposed
[feature, seq] layout so that the conv shift lands on the free dim and all
matmuls contract over the partition dim.

Schedule: the QKV projection streams hidden-state chunks; conv/rope/v-transpose
preprocessing is interleaved per 512-column chunk so it hides under the next
chunk's matmuls. Attention runs query-block-outer with the per-block o_proj
software-pipelined one block behind, keeping the PE stream dense end to end.
"""

import os
from contextlib import ExitStack

import numpy as np

import concourse.bacc as bacc
import concourse.mybir as mybir
import concourse.tile as tile
from concourse.bass_utils import run_bass_kernel_spmd
from concourse.masks import make_identity

B, S, HID = 1, 2048, 4096
H, KV, D = 32, 8, 128
THETA = 100000.0
NCORES = 8
HPC = H // NCORES            # query heads per core = 4
QCOLS = HPC * D              # 512
WCOLS = QCOLS + 2 * D        # 768 = q(512) | k(128) | v(128)
MCH = WCOLS // 128           # 6
NKC = HID // 128             # 32
NOB = HID // 512             # 8 o_proj column chunks
SCALE = 1.0 / float(np.sqrt(D))

F32 = mybir.dt.float32
BF16 = mybir.dt.bfloat16
F32R = mybir.dt.float32r
MULT = mybir.AluOpType.mult
ADD = mybir.AluOpType.add
EXP = mybir.ActivationFunctionType.Exp


def build_nc(S_=S, dt_mm="f32r"):
    """Build the single-core SPMD Bass program.

    dt_mm: 'f32' (exact, 4 cyc/row), 'f32r' (fp32 data, fast PE path),
           'bf16' (bf16 matmul data everywhere).
    """
    dt_d = {"f32": F32, "f32r": F32R, "bf16": BF16}[dt_mm]
    dt_stair = BF16 if dt_mm == "bf16" else F32
    NB = S_ // 512               # query blocks of 512
    SBLK = S_ // 128             # 128-blocks along seq

    nc = bacc.Bacc("TRN2", target_bir_lowering=False)
    hT = nc.dram_tensor("hT", [NKC, NB, 128, 512], dt_d, kind="ExternalInput")
    wqkvT = nc.dram_tensor("wqkvT", [HID, WCOLS], dt_d, kind="ExternalInput")
    woT = nc.dram_tensor("woT", [HPC, D, HID], dt_d, kind="ExternalInput")
    cosF = nc.dram_tensor("cosF", [128, S_], F32, kind="ExternalInput")
    sinF = nc.dram_tensor("sinF", [128, S_], F32, kind="ExternalInput")
    convk = nc.dram_tensor("convk", [128, 2], F32, kind="ExternalInput")
    convv = nc.dram_tensor("convv", [128, 2], F32, kind="ExternalInput")
    ones_c = nc.dram_tensor("ones_c", [128, 1], dt_d, kind="ExternalInput")
    ones_r = nc.dram_tensor("ones_r", [1, 128], F32, kind="ExternalInput")
    outp = nc.dram_tensor("outp", [NOB, SBLK, 128, 512], F32,
                          kind="ExternalOutput")

    with tile.TileContext(nc) as tc, ExitStack() as top:
        persist = top.enter_context(tc.tile_pool(name="persist", bufs=1))
        # lower-left triangle: keep where col >= row
        stair = persist.tile([128, 128], dt_stair, tag="stair")
        nc.gpsimd.memset(stair[:], 1.0)
        nc.gpsimd.affine_select(
            out=stair[:], in_=stair[:], compare_op=mybir.AluOpType.is_ge,
            fill=0.0, base=0, pattern=[[1, 128]], channel_multiplier=-1)
        ones_col = persist.tile([128, 1], dt_d, tag="ones_col")
        nc.gpsimd.dma_start(ones_col[:], ones_c[:])
        ones_row = persist.tile([1, 128], F32, tag="ones_row")
        nc.gpsimd.dma_start(ones_row[:], ones_r[:])

        res1 = top.enter_context(tc.tile_pool(name="res1", bufs=1))
        qTr = [res1.tile([128, S_], dt_d, tag=f"qTr{h}", name=f"qTr{h}")
               for h in range(HPC)]
        kTr = res1.tile([128, S_], dt_d, tag="kTr")
        vS = res1.tile([128, S_], dt_d, tag="vS")

        # ============ phase 1: QKV projection + fused preproc ============
        with ExitStack() as ph1:
            trig = ph1.enter_context(tc.tile_pool(name="trig", bufs=1))
            wp = ph1.enter_context(tc.tile_pool(name="wp", bufs=1))
            hp = ph1.enter_context(tc.tile_pool(name="hp", bufs=4))
            ckp = ph1.enter_context(tc.tile_pool(name="ckp", bufs=12))
            tp = ph1.enter_context(tc.tile_pool(name="tp", bufs=2))
            psq = ph1.enter_context(tc.tile_pool(name="psq", bufs=6,
                                                 space="PSUM"))
            pst = ph1.enter_context(tc.tile_pool(name="pst", bufs=2,
                                                 space="PSUM"))
            ck_sb = trig.tile([128, 2], F32, tag="ck_sb")
            cv_sb = trig.tile([128, 2], F32, tag="cv_sb")
            nc.gpsimd.dma_start(ck_sb[:], convk[:])
            nc.gpsimd.dma_start(cv_sb[:], convv[:])
            ident = trig.tile([128, 128], F32, tag="ident")
            make_identity(nc, ident[:])
            cos_sb = trig.tile([128, S_], F32, tag="cos_sb")
            sin_sb = trig.tile([128, S_], F32, tag="sin_sb")
            nc.gpsimd.dma_start(cos_sb[:], cosF[:])
            nc.gpsimd.dma_start(sin_sb[:], sinF[:])

            def rope_chunk(dst, x, n):
                """dst[:, n*512:+512] = rope(x); x is [128,512] fp32 chunk."""
                sl = slice(n * 512, (n + 1) * 512)
                t1 = tp.tile([128, 512], F32, tag="r1", name=f"r1_{n}")
                t2 = tp.tile([128, 512], F32, tag="r2", name=f"r2_{n}")
                nc.vector.tensor_copy(t2[0:64, :], x[64:128, :])
                nc.vector.tensor_copy(t2[64:128, :], x[0:64, :])
                nc.vector.tensor_tensor(t1[:], x[:], cos_sb[:, sl], MULT)
                nc.vector.tensor_tensor(t2[:], t2[:], sin_sb[:, sl], MULT)
                nc.vector.tensor_tensor(dst[:, sl], t1[:], t2[:], ADD)

            def conv_chunk(x, xprev, cc, n):
                """width-2 causal conv on a [128,512] chunk; returns tile."""
                t2 = tp.tile([128, 512], F32, tag="r2", name=f"cv2_{n}")
                dst = tp.tile([128, 512], F32, tag="cv1", name=f"cv1_{n}")
                nc.vector.tensor_scalar_mul(t2[:], x[:], cc[:, 0:1])
                nc.vector.scalar_tensor_tensor(
                    out=dst[:, 1:512], in0=x[:, 1:512], scalar=cc[:, 1:2],
                    in1=t2[:, 0:511], op0=MULT, op1=ADD)
                if xprev is None:
                    nc.vector.tensor_scalar_mul(dst[:, 0:1], x[:, 0:1],
                                                cc[:, 1:2])
                else:
                    tb = tp.tile([128, 1], F32, tag="cvp", name=f"cvp_{n}")
                    nc.vector.tensor_scalar_mul(tb[:], xprev[:, 511:512],
                                                cc[:, 0:1])
                    nc.vector.scalar_tensor_tensor(
                        out=dst[:, 0:1], in0=x[:, 0:1], scalar=cc[:, 1:2],
                        in1=tb[:], op0=MULT, op1=ADD)
                return dst

            w_sb = [wp.tile([128, WCOLS], dt_d, tag=f"w{k}", name=f"w{k}")
                    for k in range(NKC)]
            kprev = [None, None]  # previous raw chunks of k, v for conv edge
            for n in range(NB):
                psums = [psq.tile([128, 512], F32, tag="qkps",
                                  name=f"qkps{n}_{m}") for m in range(MCH)]
                for k in range(NKC):
                    if n == 0:
                        weng = nc.scalar if k % 2 == 0 else nc.sync
                        weng.dma_start(w_sb[k][:],
                                       wqkvT[k * 128:(k + 1) * 128, :])
                    ht = hp.tile([128, 512], dt_d, tag="ht", name=f"ht{n}_{k}")
                    heng = nc.sync if k % 2 == 0 else nc.scalar
                    heng.dma_start(ht[:], hT[k, n, :, :])
                    for m in range(MCH):
                        nc.tensor.matmul(
                            psums[m][:], lhsT=w_sb[k][:, m * 128:(m + 1) * 128],
                            rhs=ht[:], start=(k == 0), stop=(k == NKC - 1))
                chunks = [None] * MCH

                def evac(m):
                    ch = ckp.tile([128, 512], F32, tag="qkvch",
                                  name=f"ch{n}_{m}")
                    nc.vector.tensor_copy(ch[:], psums[m][:])
                    chunks[m] = ch

                # Evacuate PSUM banks promptly so the next chunk's matmuls
                # get banks back; on the last chunk interleave dependency-
                # first (k, q0 early) to shorten the phase transition.
                if n < NB - 1:
                    for m in (HPC, 0, HPC + 1, 1, 2, 3):
                        evac(m)
                    kc = conv_chunk(chunks[HPC], kprev[0], ck_sb, n)
                    rope_chunk(kTr, kc, n)
                    rope_chunk(qTr[0], chunks[0], n)
                    vc = conv_chunk(chunks[HPC + 1], kprev[1], cv_sb, n)
                    for jj in range(4):
                        jb = 4 * n + jj
                        pt = pst.tile([128, 128], F32, tag="pt",
                                      name=f"pt{jb}")
                        nc.tensor.transpose(
                            pt[:], vc[:, jj * 128:(jj + 1) * 128], ident[:])
                        nc.vector.tensor_copy(
                            vS[:, jb * 128:(jb + 1) * 128], pt[:])
                    for h in range(1, HPC):
                        rope_chunk(qTr[h], chunks[h], n)
                else:
                    # bank-aware order: evac m=0 frees the bank the first
                    # scores tile reuses; k (m=4) unblocks rope k
                    evac(0)
                    evac(HPC)
                    kc = conv_chunk(chunks[HPC], kprev[0], ck_sb, n)
                    rope_chunk(kTr, kc, n)
                    rope_chunk(qTr[0], chunks[0], n)
                    evac(1)
                    evac(HPC + 1)
                    vc = conv_chunk(chunks[HPC + 1], kprev[1], cv_sb, n)
                    evac(2)
                    evac(3)
                    for jj in range(4):
                        jb = 4 * n + jj
                        pt = pst.tile([128, 128], F32, tag="pt",
                                      name=f"pt{jb}")
                        nc.tensor.transpose(
                            pt[:], vc[:, jj * 128:(jj + 1) * 128], ident[:])
                        nc.vector.tensor_copy(
                            vS[:, jb * 128:(jb + 1) * 128], pt[:])
                    for h in range(1, HPC):
                        rope_chunk(qTr[h], chunks[h], n)
                kprev = [chunks[HPC], chunks[HPC + 1]]

        # ============ phases 2+3: attention + pipelined o_proj ===========
        with ExitStack() as ph34:
            big = ph34.enter_context(tc.tile_pool(name="big", bufs=1))
            exp_p = ph34.enter_context(tc.tile_pool(name="exp_p", bufs=4))
            smalls = ph34.enter_context(tc.tile_pool(name="smalls", bufs=2))
            stg = ph34.enter_context(tc.tile_pool(name="stg", bufs=4))
            ps_s = ph34.enter_context(tc.tile_pool(name="ps_s", bufs=2,
                                                   space="PSUM"))
            ps_o = ph34.enter_context(tc.tile_pool(name="ps_o", bufs=2,
                                                   space="PSUM"))
            ps_d = ph34.enter_context(tc.tile_pool(name="ps_d", bufs=1,
                                                   space="PSUM"))
            ps_b = ph34.enter_context(tc.tile_pool(name="ps_b", bufs=1,
                                                   space="PSUM"))
            ps_p = ph34.enter_context(tc.tile_pool(name="ps_p", bufs=2,
                                                   space="PSUM"))
            AT = [big.tile([128, S_], dt_d, tag=f"AT{h}", name=f"AT{h}")
                  for h in range(HPC)]
            woT_sb = []
            for h in range(HPC):
                wt = big.tile([128, HID], dt_d, tag=f"woT{h}", name=f"woT{h}")
                nc.scalar.dma_start(wt[:], woT[h, :, :])
                woT_sb.append(wt)

            def oproj_block(bi):
                for nb in range(NOB):
                    for sb in range(4 * bi, 4 * bi + 4):
                        pp = ps_p.tile([128, 512], F32, tag="pp",
                                       name=f"pp{nb}_{sb}")
                        for h in range(HPC):
                            nc.tensor.matmul(
                                pp[:], lhsT=AT[h][:, sb * 128:(sb + 1) * 128],
                                rhs=woT_sb[h][:, nb * 512:(nb + 1) * 512],
                                start=(h == 0), stop=(h == HPC - 1))
                        ot = stg.tile([128, 512], F32, tag="ot",
                                      name=f"ot{nb}_{sb}")
                        nc.vector.tensor_copy(ot[:], pp[:])
                        oeng = nc.sync if (nb + sb) % 2 == 0 else nc.gpsimd
                        oeng.dma_start(outp[nb, sb, :, :], ot[:])

            def norm_head(job):
                h, bi, po, pd = job
                dsb = smalls.tile([1, 512], F32, tag="dsb",
                                  name=f"dsb{h}_{bi}")
                nc.vector.tensor_copy(dsb[:], pd[:])
                rec = smalls.tile([1, 512], F32, tag="rec",
                                  name=f"rec{h}_{bi}")
                scr = smalls.tile([1, 512], F32, tag="scr",
                                  name=f"scr{h}_{bi}")
                nc.vector.reciprocal_approx_accurate(rec[:], dsb[:], scr[:])
                pb = ps_b.tile([128, 512], F32, tag="pb", name=f"pb{h}_{bi}")
                nc.tensor.matmul(pb[:], lhsT=ones_row[:], rhs=rec[:],
                                 start=True, stop=True)
                pbs = smalls.tile([128, 512], F32, tag="pbs",
                                  name=f"pbs{h}_{bi}")
                nc.vector.tensor_copy(pbs[:], pb[:])
                nc.vector.tensor_tensor(AT[h][:, bi * 512:(bi + 1) * 512],
                                        po[:], pbs[:], MULT)

            pending_norm = None
            for bi in range(NB):
                for h in range(HPC):
                    po = ps_o.tile([128, 512], F32, tag="po", name=f"po{h}_{bi}")
                    pd = ps_d.tile([1, 512], F32, tag="pd", name=f"pd{h}_{bi}")
                    nj = 4 * bi + 4
                    for jb in range(nj):
                        t = jb - 4 * bi
                        lo = 128 * t if t > 0 else 0   # valid query range start
                        ps = ps_s.tile([128, 512], F32, tag="ps",
                                       name=f"ps{h}_{bi}_{jb}")
                        nc.tensor.matmul(
                            ps[:, lo:512],
                            lhsT=kTr[:, jb * 128:(jb + 1) * 128],
                            rhs=qTr[h][:, bi * 512 + lo:(bi + 1) * 512],
                            start=True, stop=True)
                        ex = exp_p.tile([128, 512], dt_d, tag="ex",
                                        name=f"ex{h}_{bi}_{jb}")
                        nc.scalar.activation(ex[:, lo:512], ps[:, lo:512],
                                             EXP, scale=SCALE)
                        if t >= 0:
                            nc.vector.tensor_tensor(
                                ex[:, 128 * t:128 * t + 128],
                                ex[:, 128 * t:128 * t + 128], stair[:], MULT)
                        nc.tensor.matmul(po[:, lo:512],
                                         lhsT=vS[:, jb * 128:(jb + 1) * 128],
                                         rhs=ex[:, lo:512], start=(jb == 0),
                                         stop=(jb == nj - 1))
                        nc.tensor.matmul(pd[:, lo:512], lhsT=ones_col[:],
                                         rhs=ex[:, lo:512], start=(jb == 0),
                                         stop=(jb == nj - 1))
                    if pending_norm is not None:
                        norm_head(pending_norm)
                    pending_norm = (h, bi, po, pd)
                if bi > 0:
                    oproj_block(bi - 1)
            norm_head(pending_norm)
            oproj_block(NB - 1)

    nc.compile()
    return nc


def host_prep(hidden_states, w_pack, w_o, conv_k, conv_v, S_=S, dt_mm="f32r"):
    """Slice/transpose full inputs into 8 per-core input maps."""
    if dt_mm == "bf16":
        import ml_dtypes
        dt_np = ml_dtypes.bfloat16
    else:
        dt_np = np.float32

    hid2 = np.asarray(hidden_states).reshape(S_, HID)
    # chunk-major pack: [NKC, NB, 128, 512] so each DMA tile is contiguous
    hTv = np.ascontiguousarray(
        hid2.T.reshape(NKC, 128, S_ // 512, 512).transpose(0, 2, 1, 3)
    ).astype(dt_np, copy=False)

    inv = 1.0 / (THETA ** (np.arange(0, D, 2, dtype=np.float32) / D))  # [64]
    t = np.arange(S_, dtype=np.float32)
    freqs = t[:, None] * inv[None, :]                                   # [S, 64]
    cos = np.cos(freqs).T.astype(np.float32)                            # [64, S]
    sin = np.sin(freqs).T.astype(np.float32)
    cosF = np.ascontiguousarray(np.concatenate([cos, cos], 0))          # [128, S]
    sinF = np.ascontiguousarray(np.concatenate([-sin, sin], 0))

    w_pack = np.asarray(w_pack)
    w_o = np.asarray(w_o)
    conv_k = np.asarray(conv_k)
    conv_v = np.asarray(conv_v)

    in_maps = []
    for c in range(NCORES):
        qw = w_pack[c * QCOLS:(c + 1) * QCOLS]                 # [512, HID]
        kw = w_pack[H * D + c * D:H * D + (c + 1) * D]         # [128, HID]
        vw = w_pack[H * D + KV * D + c * D:H * D + KV * D + (c + 1) * D]
        wqkvT = np.ascontiguousarray(
            np.concatenate([qw, kw, vw], 0).T).astype(dt_np, copy=False)
        woT = np.ascontiguousarray(
            w_o[:, c * QCOLS:(c + 1) * QCOLS].T.reshape(HPC, D, HID)
        ).astype(dt_np, copy=False)
        in_maps.append(dict(
            hT=hTv, wqkvT=wqkvT, woT=woT, cosF=cosF, sinF=sinF,
            ones_c=np.ones((128, 1), dt_np),
            ones_r=np.ones((1, 128), np.float32),
            convk=np.ascontiguousarray(
                np.broadcast_to(conv_k[c], (128, 2))).astype(np.float32),
            convv=np.ascontiguousarray(
                np.broadcast_to(conv_v[c], (128, 2))).astype(np.float32),
        ))
    return in_maps


def gather(results):
    """Sum per-core chunk-major partials -> full [B, S, HID] output."""
    acc = results[0]["outp"].astype(np.float32)
    for c in range(1, NCORES):
        acc = acc + results[c]["outp"]
    # [NOB, SBLK, 128, 512] chunk-major -> [S, HID]
    out = acc.transpose(1, 2, 0, 3).reshape(S, HID)
    return np.ascontiguousarray(out).reshape(B, S, HID)


_NC_CACHE = {}


def _get_nc(S_=S, dt_mm="f32r"):
    key = (S_, dt_mm)
    if key not in _NC_CACHE:
        _NC_CACHE[key] = build_nc(S_, dt_mm)
    return _NC_CACHE[key]


def kernel(**inputs):
    dt_mm = os.environ.get("KERNEL_DT_MM", "f32r")
    nc = _get_nc(S, dt_mm)
    in_maps = host_prep(
        inputs["hidden_states"], inputs["w_pack"], inputs["w_o"],
        inputs["conv_k"], inputs["conv_v"], S, dt_mm)
    res = run_bass_kernel_spmd(nc, in_maps, core_ids=list(range(NCORES)))
    return gather(res.results)
